# revision 25
# baseline (speedup 1.0000x reference)
"""Causal multi-head attention with RoPE on 8 Trainium2 NeuronCores.

Problem: x[1,4096,1024], 16 heads, head_dim 64, causal, RoPE(theta=1e4),
Q/K/V/O projections. Sharding: 2 heads per core (head-parallel). Each core:
  - computes Q^T,K^T (transposed, RoPE'd, bf16) and V (natural, bf16) for its
    2 heads
  - flash-style causal attention with scores kept transposed (S^T[k,q]) so
    P^T feeds the P@V matmul directly; softmax denominator comes from an
    appended ones-column in V (M=65 matmul); no max-subtraction needed
    (scores ~ N(0,1) -> exp never overflows)
  - o_proj partial (its 128 ctx columns x full Wo) -> out^T[1024,4096] bf16
Host: transposes x / weight slices (cast bf16), builds RoPE cos/sin tables
(f32), sums the 8 partial outputs (f32) and transposes back.

Perf notes (measured on HW, 327us baseline -> ~280us):
  - softmax 1/den: DVE Reciprocal is an 8-pass iterative divide (8 cyc per
    free elem) so a [1,1024] recip costs 6.8us; we DMA-bounce the dens to a
    [128,8] layout (SBUF->SBUF, linear walk), recip there (~200ns), DMA back.
  - o_part is written bf16 (halves output DMA; host sums in f32).
  - V transpose in bf16 via PE (1 cyc/row); psum evacuations are split
    between DVE and ScalarE by chunk phase (ScalarE is exp-saturated in late
    chunks, idle early; DVE gates the proj chain early).
  - K/V projection is prefetched one chunk ahead for early (PE-thin) chunks
    and computed just-in-time inside late (exp-bound) chunks.
  - dummy warm-keeper matmuls bracket the final tail so the HAM clock gate
    doesn't halve the PE clock during the last o_proj.
  - fp8 DoubleRow for P@V was tried and REVERTED: the rhs still streams one
    element/cycle in this layout (measured ~700ns vs 2x213ns bf16), and the
    exp->fp8 path NaN'd. reciprocal_approx_fast (custom DVE op) also NaN's
    on this NRT path - do not use.

Matmul operands are bf16 (1 cyc/row on PE); accumulation is f32 in PSUM.
"""
import os
import sys

sys.path.insert(0, "/opt/trn_rl_repo")

import numpy as np

D_MODEL = 1024
N_HEADS = 16
SEQ = 4096
DHEAD = 64
THETA = 10000.0
N_CORES = 8
CHUNK = 512          # seq chunk = q-block width
NKB = SEQ // 128     # 32 k-blocks of 128


def _build_program():
    from contextlib import ExitStack

    import concourse.bass as bass
    import concourse.mybir as mybir
    import concourse.tile as tile
    from concourse import bacc
    from concourse.masks import make_identity

    F32 = mybir.dt.float32
    F32R = mybir.dt.float32r
    BF16 = mybir.dt.bfloat16
    FP8 = mybir.dt.float8e4
    DR = mybir.MatmulPerfMode.DoubleRow
    AF = mybir.ActivationFunctionType

    nc = bacc.Bacc()

    xt_d = nc.dram_tensor("xt", [D_MODEL, SEQ], BF16, kind="ExternalInput")
    wq_d = nc.dram_tensor("wq", [D_MODEL, 128], BF16, kind="ExternalInput")
    wk_d = nc.dram_tensor("wk", [D_MODEL, 128], BF16, kind="ExternalInput")
    wv_d = nc.dram_tensor("wv", [D_MODEL, 128], BF16, kind="ExternalInput")
    wo_d = nc.dram_tensor("wo", [128, D_MODEL], BF16, kind="ExternalInput")
    t1_d = nc.dram_tensor("t1", [128, SEQ], F32, kind="ExternalInput")
    t2_d = nc.dram_tensor("t2", [128, SEQ], F32, kind="ExternalInput")
    m1_d = nc.dram_tensor("m1", [128, 128], BF16, kind="ExternalInput")
    m2_d = nc.dram_tensor("m2", [128, 128], BF16, kind="ExternalInput")
    o_d = nc.dram_tensor("o_part", [D_MODEL, SEQ], BF16, kind="ExternalOutput")

    NCHUNK = SEQ // CHUNK  # 8

    with tile.TileContext(nc) as tc:
        with nc.allow_low_precision(reason="bf16 compute; f32 accumulate"), \
             ExitStack() as ctx:
            const = ctx.enter_context(tc.tile_pool(name="const", bufs=1))
            persist = ctx.enter_context(tc.tile_pool(name="persist", bufs=1))
            work = ctx.enter_context(tc.tile_pool(name="work", bufs=1))
            psum = ctx.enter_context(tc.tile_pool(name="psum", bufs=1, space="PSUM"))

            m1_sb = const.tile([128, 128], BF16, name="m1_sb", tag="m1_sb")
            nc.sync.dma_start(m1_sb[:], m1_d[:])
            # PE warm-up: dense dummy matmuls during the DMA-bound head keep
            # the HAM clock-gate at full rate before real work arrives. m1 is
            # the first DMA issued so the PE lights up as early as possible.
            warm_ps = psum.tile([128, 128], F32, name="warm_ps", tag="misc",
                                bufs=2)
            for _ in range(48):
                nc.tensor.matmul(warm_ps[:], m1_sb[:], m1_sb[:],
                                 start=True, stop=True)
            ident = const.tile([128, 128], F32, name="ident", tag="ident")
            make_identity(nc, ident[:])
            identb = const.tile([128, 128], BF16, name="identb", tag="identb")
            nc.vector.tensor_copy(identb[:], ident[:])
            # ones2[h, :] selects head h's 64-row block: one matmul broadcasts
            # both heads' per-column 1/den [2,512] -> [128,512]. Built via a
            # linear SBUF->SBUF DMA (engines can't write at partition 1).
            o2flat = const.tile([1, 256], F32, name="o2flat", tag="o2flat")
            nc.vector.memset(o2flat[:], 0.0)
            nc.vector.memset(o2flat[:, 0:64], 1.0)
            nc.vector.memset(o2flat[:, 192:256], 1.0)
            o2r = const.tile([1, 256], F32R, name="o2r", tag="o2r")
            nc.vector.tensor_copy(o2r[:], o2flat[:])
            ones2 = const.tile([2, 128], F32R, name="ones2", tag="ones2")
            nc.sync.dma_start(ones2[:], o2r[:])
            onescol = const.tile([128, 2, 1], BF16, name="onescol", tag="onescol")
            nc.vector.memset(onescol[:], 1.0)
            zbias = const.tile([128, 1], F32, name="zbias", tag="zbias")
            nc.vector.memset(zbias[:], 0.0)

            m2_sb = const.tile([128, 128], BF16, name="m2_sb", tag="m2_sb")
            nc.sync.dma_start(m2_sb[:], m2_d[:])
            wo_sb = const.tile([128, D_MODEL], BF16, name="wo_sb", tag="wo_sb")

            wq_sb = const.tile([128, 8, 128], BF16, name="wq_sb", tag="wq_sb")
            wk_sb = const.tile([128, 8, 128], BF16, name="wk_sb", tag="wk_sb")
            wv_sb = const.tile([128, 8, 128], BF16, name="wv_sb", tag="wv_sb")
            w_sb = {"q": wq_sb, "k": wk_sb, "v": wv_sb}

            def load_w(d_t, sb):
                # d_t [1024, 128] viewed as [p 128, i 8, col 128]
                nc.sync.dma_start(
                    sb[:], d_t.rearrange("(i p) c -> p i c", i=8))

            load_w(wq_d, wq_sb)

            qt = [None] * NCHUNK   # Q^T chunks [128, 512] bf16 (RoPE'd, d-perm)
            kt = [None] * NCHUNK
            vsb = [None] * NKB     # V natural per k-block [128, 2, 65] bf16
            xts_c = {}
            xtsq_c = {}
            tabs_c = {}

            def load_chunk(c):
                cs = slice(c * CHUNK, (c + 1) * CHUNK)
                t = work.tile([128, 8, CHUNK], BF16, name=f"xt_{c}",
                              tag="xt", bufs=3)
                nc.sync.dma_start(
                    t[:], xt_d.rearrange("(i p) s -> p i s", i=8)[:, :, cs])
                xts_c[c] = t
                t1c = const.tile([128, CHUNK], F32, name=f"t1c{c}", tag=f"t1c{c}")
                nc.sync.dma_start(t1c[:], t1_d[:, cs])
                t2c = const.tile([128, CHUNK], F32, name=f"t2c{c}", tag=f"t2c{c}")
                nc.sync.dma_start(t2c[:], t2_d[:, cs])
                tabs_c[c] = (t1c, t2c)

            def proj(kind, c):
                w = w_sb[kind]
                ps = psum.tile([128, CHUNK], F32, name=f"{kind}ps{c}",
                               tag="misc", bufs=2)
                xts = xts_c[c]
                for i in range(8):
                    nc.tensor.matmul(ps[:], w[:, i, :], xts[:, i, :],
                                     start=(i == 0), stop=(i == 7))
                if kind in ("q", "k"):
                    t1c, t2c = tabs_c[c]
                    p1 = work.tile([128, CHUNK], BF16, name=f"p1_{kind}{c}",
                                   tag="p1", bufs=3)
                    nc.vector.tensor_mul(p1[:], t1c[:], ps[:])
                    p2 = work.tile([128, CHUNK], BF16, name=f"p2_{kind}{c}",
                                   tag="p2", bufs=3)
                    nc.vector.tensor_mul(p2[:], t2c[:], ps[:])
                    rp = psum.tile([128, CHUNK], F32, name=f"rp_{kind}{c}",
                                   tag="misc", bufs=2)
                    nc.tensor.matmul(rp[:], m1_sb[:], p1[:],
                                     start=True, stop=False)
                    nc.tensor.matmul(rp[:], m2_sb[:], p2[:],
                                     start=False, stop=True)
                    dst = persist.tile([128, CHUNK], BF16,
                                       name=f"{kind}t{c}", tag=f"{kind}t{c}")
                    # early chunks: ScalarE is idle and DVE gates the proj
                    # chain; late chunks: ScalarE is exp-saturated
                    if c <= 4:
                        nc.scalar.copy(dst[:], rp[:])
                    else:
                        nc.vector.tensor_copy(dst[:], rp[:])
                    if kind == "q":
                        qt[c] = dst
                    else:
                        kt[c] = dst
                else:
                    vt = work.tile([128, CHUNK], BF16, name=f"vt{c}",
                                   tag="vt", bufs=2)
                    if c <= 4:
                        nc.scalar.copy(vt[:], ps[:])
                    else:
                        nc.vector.tensor_copy(vt[:], ps[:])
                    vt_c[c] = vt

            vt_c = {}

            def vtrans(c, j):
                kb = c * 4 + j
                vn = psum.tile([128, 128], BF16, name=f"vn{kb}",
                               tag="misc", bufs=2)
                nc.tensor.transpose(vn[:],
                                    vt_c[c][:, j * 128:(j + 1) * 128],
                                    identb[:])
                vb = persist.tile([128, 2, 65], BF16, name=f"v{kb}",
                                  tag=f"v{kb}")
                if c <= 5:
                    nc.scalar.copy(vb[:, :, 0:64],
                                   vn[:].rearrange("p (h d) -> p h d", h=2))
                else:
                    nc.vector.tensor_copy(
                        vb[:, :, 0:64],
                        vn[:].rearrange("p (h d) -> p h d", h=2))
                nc.vector.tensor_copy(vb[:, :, 64:65], onescol[:])
                vsb[kb] = vb

            # deferred per-q-block state for the pipelined tail
            pend = {}
            pend_dsq = {}
            ctx_live = {}
            pend_ctx = {}   # qb -> list of (kbs, [ph_h0, ph_h1]) awaiting ctx MMs

            def attn_pair(qb, p0):
                # Scores for both heads of one k-block go into ONE psum tile
                # [128, 2, 512] (h-major columns): the h1 matmul shares the h0
                # matmul's buffer dependency, so the tile scheduler emits the
                # two K=64 tile_position-packed MMs back-to-back and they run
                # CONCURRENTLY on the PE (row groups 0-63 / 64-127). With the
                # old per-head tiles the h1 MM carried a semaphore wait on the
                # previous pair's h1 exp, which landed mid-stream of the h0 MM
                # and serialized the pair (trace: 317+216ns vs 317+4ns).
                # Diagonal blocks j=1,2 are column-trimmed (the first j*128
                # q-columns are fully masked); j=3 stays full so the stop=True
                # ctx matmul covers the whole accumulation region.
                nkb = 4 * (qb + 1)
                if qb not in ctx_live:
                    ctx_live[qb] = [
                        psum.tile([65, CHUNK], F32, name=f"ctx_{qb}_{h}",
                                  tag="ctx", bufs=2)
                        for h in (0, 1)]
                    pend_ctx[qb] = []
                kbs = list(range(p0, min(p0 + 2, nkb)))
                s2s = []
                for kb in kbs:
                    j = kb - 4 * qb
                    off = j * 128 if j in (1, 2, 3) else 0
                    s2 = psum.tile([128, 2, CHUNK], F32,
                                   name=f"s2_{qb}_{kb}", tag="scores",
                                   bufs=2)
                    for h in (0, 1):
                        nc.tensor.matmul(
                            s2[:, h, off:],
                            kt[kb // 4][h * 64:(h + 1) * 64,
                                        (kb % 4) * 128:(kb % 4) * 128 + 128],
                            qt[qb][h * 64:(h + 1) * 64, off:],
                            start=True, stop=True,
                            tile_position=(h * 64, 0))
                    s2s.append((kb, off, s2))
                entry = []
                for kb, off, s2 in s2s:
                    ph = work.tile([128, 2, CHUNK], BF16,
                                   name=f"ph_{qb}_{kb}", tag="p2h", bufs=8)
                    nc.scalar.activation(ph[:, :, off:], s2[:, :, off:],
                                         AF.Exp, bias=zbias[:], scale=0.125)
                    sl = []
                    if kb >= 4 * qb:  # diagonal: zero k_global > q_global
                        ncols = CHUNK - off
                        for h in (0, 1):
                            pm = work.tile([128, CHUNK], BF16,
                                           name=f"pm_{qb}_{kb}_{h}",
                                           tag="phm", bufs=6)
                            nc.gpsimd.affine_select(
                                out=pm[:, 0:ncols],
                                in_=ph[:, h, off:],
                                pattern=[[1, ncols]],
                                compare_op=mybir.AluOpType.is_ge,
                                fill=0.0,
                                base=-(kb - 4 * qb) * 128 + off,
                                channel_multiplier=-1)
                            sl.append((pm, None, 0, ncols, off))
                    else:
                        for h in (0, 1):
                            sl.append((ph, h, 0, CHUNK, 0))
                    entry.append((kb, sl))
                pend_ctx[qb].append(entry)
                # emit ctx for the PREVIOUS pending pair (depth-1 pipeline)
                if len(pend_ctx[qb]) > 1:
                    _emit_ctx_entry(qb, pend_ctx[qb].pop(0))

            def _emit_ctx_entry(qb, entry):
                nkb = 4 * (qb + 1)
                ctx_ps = ctx_live[qb]
                for h in (0, 1):
                    for kb, sl in entry:
                        tile_, hsel, lo, hi, ooff = sl[h]
                        rhs = (tile_[:, lo:hi] if hsel is None
                               else tile_[:, hsel, lo:hi])
                        nc.tensor.matmul(
                            ctx_ps[h][:, ooff:],
                            vsb[kb][:, h, :],
                            rhs,
                            start=(kb == 0), stop=(kb == nkb - 1),
                            skip_group_check=(ooff > 0))

            def attn_finish(qb):
                # flush remaining pending ctx pairs, evacuate ctx psum to SBUF
                # (frees the psum slots for the next q-block immediately), then
                # 1/den = exp(-ln(den)) on ScalarE (same act-table set as the
                # softmax exp), written bf16 for the broadcast matmul.
                for entry_prev in pend_ctx.pop(qb):
                    _emit_ctx_entry(qb, entry_prev)
                ctx_ps = ctx_live.pop(qb)
                ctxs = []
                den = work.tile([1, 2 * CHUNK], F32, name=f"den{qb}",
                                tag="den", bufs=2)
                # den rows first: starts the reciprocal DMA round-trip while
                # the (bigger) ctx evacuations still run; the two heads go to
                # different engines so the copies overlap
                nc.vector.tensor_copy(den[:, 0:CHUNK], ctx_ps[0][64:65, :])
                nc.scalar.copy(den[:, CHUNK:2 * CHUNK], ctx_ps[1][64:65, :])
                # the [1,1024] reciprocal is free-size bound (8 ALU passes
                # over the free dim); bounce via DMA to [128,8] where the
                # same op costs ~70ns, then DMA back. (Tried ScalarE
                # exp(-ln(x)) for the final chunk instead: walrus picks the
                # exp_and_others act table set, so the Ln costs TWO mid-tail
                # ACT_TABLE_LOADs (~2.6us) - a net loss. Do not repeat.)
                dsq = work.tile([128, 8], F32, name=f"dsq{qb}",
                                tag="dsq", bufs=2)
                nc.sync.dma_start(dsq[:], den[:])
                pend_dsq[qb] = dsq
                rsq = work.tile([128, 8], F32R, name=f"rsq{qb}",
                                tag="rsq", bufs=2)
                nc.vector.reciprocal(rsq[:], dsq[:])
                rec = work.tile([2, CHUNK], F32R, name=f"rec{qb}",
                                tag="rec", bufs=2)
                nc.sync.dma_start(rec[:], rsq[:])
                flat = False
                for h in (0, 1):
                    cs_ = work.tile([64, CHUNK], F32, name=f"ctxs{qb}{h}",
                                    tag="ctxs", bufs=4)
                    nc.vector.tensor_copy(cs_[:], ctx_ps[h][0:64, :])
                    ctxs.append(cs_)
                pend[qb] = (ctxs, rec, flat)

            ctxn_live = {}

            def tail_norm(qb):
                ctxs, rec, flat = pend.pop(qb)
                ctxn = work.tile([128, CHUNK], BF16, name=f"ctxn{qb}",
                                 tag="ctxn", bufs=2)
                bc = psum.tile([128, CHUNK], F32, name=f"bc{qb}",
                               tag="misc", bufs=2)
                if flat:
                    # rec is [1, 2*CHUNK]: broadcast via two accumulated K=1
                    # matmuls (o2r halves select each head's 64 rows)
                    nc.tensor.matmul(bc[:], o2r[:, 0:128], rec[:, 0:CHUNK],
                                     start=True, stop=False)
                    nc.tensor.matmul(bc[:], o2r[:, 128:256], rec[:, CHUNK:],
                                     start=False, stop=True)
                else:
                    nc.tensor.matmul(bc[:], ones2[:], rec[:],
                                     start=True, stop=True)
                for h in (0, 1):
                    nc.vector.tensor_mul(ctxn[h * 64:(h + 1) * 64, :],
                                         ctxs[h][:], bc[h * 64:(h + 1) * 64, :])
                ctxn_live[qb] = ctxn

            osb8_live = {}

            def get_osb8(qb):
                if qb not in osb8_live:
                    osb8_live[qb] = work.tile([128, 8, CHUNK], BF16,
                                              name=f"osb8_{qb}", tag="osb8",
                                              bufs=2)
                return osb8_live[qb]

            def tail_oproj(qb, obs, split=False):
                ctxn = ctxn_live[qb]
                if split:
                    # endgame: per-ob half-width MMs into separate banks so
                    # DVE and ScalarE evacuate in parallel, all into ONE
                    # tile, then just 2 big DMAs — the final tail was paying
                    # ~600ns of serial Sync-engine setup per small DMA.
                    H = CHUNK // 2
                    osb8 = get_osb8(qb)
                    for ob in obs:
                        wslice = wo_sb[:, ob * 128:(ob + 1) * 128]
                        pa = psum.tile([128, H], F32, name=f"opsa{qb}{ob}",
                                       tag="misc", bufs=2)
                        pb = psum.tile([128, H], F32, name=f"opsb{qb}{ob}",
                                       tag="misc", bufs=2)
                        nc.tensor.matmul(pa[:], wslice, ctxn[:, 0:H],
                                         start=True, stop=True)
                        nc.tensor.matmul(pb[:], wslice, ctxn[:, H:],
                                         start=True, stop=True)
                        nc.vector.tensor_copy(osb8[:, ob, 0:H], pa[:])
                        nc.scalar.copy(osb8[:, ob, H:], pb[:])
                        if ob == 3:
                            nc.sync.dma_start(
                                o_d[0:512, qb * CHUNK:(qb + 1) * CHUNK]
                                .rearrange("(ob p) s -> p ob s", ob=4),
                                osb8[:, 0:4, :])
                    nc.sync.dma_start(
                        o_d[512:1024, qb * CHUNK:(qb + 1) * CHUNK]
                        .rearrange("(ob p) s -> p ob s", ob=4),
                        osb8[:, 4:8, :])
                    return
                # batched: evacuate all obs into one [128,8,512] tile and
                # issue just TWO output DMAs per chunk (each dma_start costs
                # ~670ns of serial sync-engine setup; 8 small DMAs/chunk had
                # the sync sequencer 60% busy)
                osb8 = get_osb8(qb)
                for ob in obs:
                    wslice = wo_sb[:, ob * 128:(ob + 1) * 128]
                    o_ps = psum.tile([128, CHUNK], F32, name=f"ops{qb}{ob}",
                                     tag="misc", bufs=2)
                    nc.tensor.matmul(o_ps[:], wslice,
                                     ctxn[:], start=True, stop=True)
                    # final chunk: ScalarE is idle, so alternating engines
                    # doubles evac throughput (the 2-slot misc ring makes the
                    # o_proj evac-limited); mid-kernel ScalarE runs exps
                    if qb == NCHUNK - 1 and ob % 2 == 1:
                        nc.scalar.copy(osb8[:, ob, :], o_ps[:])
                    else:
                        nc.vector.tensor_copy(osb8[:, ob, :], o_ps[:])
                    if ob == 3:
                        nc.sync.dma_start(
                            o_d[0:512, qb * CHUNK:(qb + 1) * CHUNK]
                            .rearrange("(ob p) s -> p ob s", ob=4),
                            osb8[:, 0:4, :])
                    elif ob == 7:
                        nc.sync.dma_start(
                            o_d[512:1024, qb * CHUNK:(qb + 1) * CHUNK]
                            .rearrange("(ob p) s -> p ob s", ob=4),
                            osb8[:, 4:8, :])

            # prologue: chunk 0's full QKV + chunk 1's q-proj. K/V of chunk
            # c>=1 is computed just-in-time DURING chunk c's early pairs (it
            # is first needed by the two diagonal pairs at the end) — this
            # moves PE work out of the PE-bound early chunks into the
            # exp-bound late chunks where the PE would otherwise idle and
            # re-throttle.
            load_chunk(0)
            proj("q", 0)
            load_w(wk_d, wk_sb)
            proj("k", 0)
            load_w(wv_d, wv_sb)
            load_chunk(1)
            proj("v", 0)
            for j in range(4):
                vtrans(0, j)
            proj("q", 1)
            # filler MMs after the prologue's real work: they plug the
            # RoPE-chain dependency stalls at the chunk 0/1 boundary so the
            # HAM clock gate never sees an idle window early on
            for _ in range(24):
                nc.tensor.matmul(warm_ps[:], m1_sb[:], m1_sb[:],
                                 start=True, stop=True)

            for c in range(NCHUNK):
                npair = 2 * (c + 1)
                # (fraction-of-chunk, unit); kv units carry a deadline slot
                U = []
                if c > 0:
                    U.append((0.0, None, lambda qb=c - 1: attn_finish(qb)))
                if c >= 3:
                    # mid/late chunks are exp-bound: compute own K/V just-in-
                    # time during the early pairs to fill otherwise-idle PE
                    kvcap = max(2 * c - 1, 1)
                    U.append((0.0, kvcap, lambda c=c: proj("k", c)))
                    U.append((0.08, kvcap, lambda c=c: proj("v", c)))
                    for j in range(4):
                        U.append((0.15 + 0.1 * j, kvcap,
                                  lambda c=c, j=j: vtrans(c, j)))
                if c <= 1:
                    # early chunks are PE-thin: prefetch next chunk's K/V
                    U.append((0.2, None, lambda c=c: proj("k", c + 1)))
                    U.append((0.3, None, lambda c=c: proj("v", c + 1)))
                    for j in range(4):
                        U.append((0.4 + 0.12 * j, None,
                                  lambda c=c, j=j: vtrans(c + 1, j)))
                if c == 1:
                    U.append((0.0, None,
                              lambda: nc.sync.dma_start(wo_sb[:], wo_d[:])))
                if c > 0:
                    U.append((0.35, None, lambda qb=c - 1: tail_norm(qb)))
                    U.append((0.5, None,
                              lambda qb=c - 1: tail_oproj(qb, range(0, 4))))
                    U.append((0.65, None,
                              lambda qb=c - 1: tail_oproj(qb, range(4, 8))))
                if 1 <= c <= 3:
                    # HAM feed: the early phase has PE-idle pockets (rope/DVE
                    # dependency stalls) long enough to re-throttle the clock
                    # (state-1 window +51..68us in the trace); LDWEIGHTS count
                    # as PE activity without touching PSUM
                    for frac in (0.15, 0.3, 0.45, 0.6, 0.75, 0.9):
                        U.append((frac, None, lambda: (
                            nc.tensor.ldweights(weights=m1_sb[:]),
                            nc.tensor.ldweights(weights=m2_sb[:]))))
                if c < NCHUNK - 2:
                    U.append((0.4, None, lambda c=c: load_chunk(c + 2)))
                if c < NCHUNK - 1 and c > 0:
                    U.append((0.8, None, lambda c=c: proj("q", c + 1)))

                slots = {}
                for frac, cap, u in U:
                    s = int(frac * npair)
                    if cap is not None:
                        s = min(s, cap)
                    slots.setdefault(s, []).append(u)
                for p in range(npair):
                    attn_pair(c, 2 * p)
                    for u in slots.pop(p, []):
                        u()
                for rest in sorted(slots):
                    for u in slots[rest]:
                        u()
            attn_finish(NCHUNK - 1)
            # keep the PE warm through the final den DMA round-trip (no real
            # PE work exists there) so the last o_proj runs at full clock.
            # Pacer MMs READ the recip chain's intermediates so the warm
            # batches are spread across the whole round-trip instead of all
            # draining instantly at the start (which let HAM re-throttle and
            # the final o_proj ran at half clock, 348ns vs ~135ns per MM).
            warm2 = psum.tile([128, 128], F32, name="warm2", tag="scores",
                              bufs=2)
            dsq7 = pend_dsq[NCHUNK - 1]
            ctxs7, rec7, _flat7 = pend[NCHUNK - 1]
            # N=128 warm MMs on a scores-tag psum tile (misc-tag would contend
            # with the o_proj/bc ring); batch sizes match each chain stage's
            # latency so the HAM activity window never sees a >3.4us idle
            for _ in range(42):
                nc.tensor.matmul(warm2[:], m1_sb[:], m1_sb[:],
                                 start=True, stop=True)
            # pacer: runs only after the first bounce DMA lands
            nc.tensor.matmul(warm2[:, 0:8], ident[:], dsq7[:],
                             start=True, stop=True)
            for _ in range(26):
                nc.tensor.matmul(warm2[:], m1_sb[:], m1_sb[:],
                                 start=True, stop=True)
            # pacer: runs only after the second bounce DMA lands
            nc.tensor.matmul(warm2[:, 0:64], ones2[:], rec7[:, 0:64],
                             start=True, stop=True)
            for _ in range(4):
                nc.tensor.matmul(warm2[:], m1_sb[:], m1_sb[:],
                                 start=True, stop=True)
            tail_norm(NCHUNK - 1)
            # fill the bc->ctxn-mul latency so HAM stays warm into the o_proj
            for _ in range(10):
                nc.tensor.matmul(warm2[:], m1_sb[:], m1_sb[:],
                                 start=True, stop=True)
            tail_oproj(NCHUNK - 1, range(0, 8))

    nc.compile()
    return nc


_PROG = None


def _get_prog():
    global _PROG
    if _PROG is None:
        _PROG = _build_program()
    return _PROG


def _make_in_maps(inputs):
    import ml_dtypes
    bf16 = ml_dtypes.bfloat16
    x = np.asarray(inputs["x"], dtype=np.float32)
    Wq = np.asarray(inputs["Wq"], dtype=np.float32)
    Wk = np.asarray(inputs["Wk"], dtype=np.float32)
    Wv = np.asarray(inputs["Wv"], dtype=np.float32)
    Wo = np.asarray(inputs["Wo"], dtype=np.float32)
    pos = np.asarray(inputs["token_positions"]).astype(np.float32)

    xt = np.ascontiguousarray(x.reshape(SEQ, D_MODEL).T.astype(bf16))

    ks = np.arange(0, DHEAD, 2, dtype=np.float32)
    inv_freq = (1.0 / np.power(np.float32(THETA), ks / np.float32(DHEAD))).astype(np.float32)
    ang = pos[:, None] * inv_freq[None, :]          # [SEQ, 32]
    cosT = np.cos(ang).T.astype(np.float32)         # [32, SEQ]
    sinT = np.sin(ang).T.astype(np.float32)
    t1 = np.ascontiguousarray(np.concatenate([cosT, -sinT, cosT, -sinT], axis=0))
    t2 = np.ascontiguousarray(np.concatenate([sinT, cosT, sinT, cosT], axis=0))

    perm = np.concatenate([np.arange(0, DHEAD, 2), np.arange(1, DHEAD, 2)])

    m1 = np.zeros((128, 128), dtype=np.float32)
    m2 = np.zeros((128, 128), dtype=np.float32)
    for m in range(128):
        if m % 64 < 32:
            m1[m, m] = 1.0
            m1[m + 32, m] = 1.0
        else:
            m2[m - 32, m] = 1.0
            m2[m, m] = 1.0
    m1 = np.ascontiguousarray(m1.astype(bf16))
    m2 = np.ascontiguousarray(m2.astype(bf16))



    in_maps = []
    for c in range(N_CORES):
        rows = np.arange(c * 128, (c + 1) * 128)
        qk_rows = np.concatenate([c * 128 + h * DHEAD + perm for h in (0, 1)])
        in_maps.append({
            "xt": xt,
            "wq": np.ascontiguousarray(Wq[qk_rows, :].T.astype(bf16)),
            "wk": np.ascontiguousarray(Wk[qk_rows, :].T.astype(bf16)),
            "wv": np.ascontiguousarray(Wv[rows, :].T.astype(bf16)),
            "wo": np.ascontiguousarray(Wo[:, rows].T.astype(bf16)),
            "t1": t1,
            "t2": t2,
            "m1": m1,
            "m2": m2,
        })
    return in_maps


def kernel(x, Wq, Wk, Wv, Wo, token_positions):
    nc = _get_prog()
    in_maps = _make_in_maps({"x": x, "Wq": Wq, "Wk": Wk, "Wv": Wv, "Wo": Wo,
                             "token_positions": token_positions})
    from concourse.bass_utils import run_bass_kernel_spmd

    res = run_bass_kernel_spmd(nc, in_maps, core_ids=list(range(N_CORES)))
    acc = res.results[0]["o_part"].astype(np.float32)
    for i in range(1, N_CORES):
        acc = acc + res.results[i]["o_part"].astype(np.float32)
    return np.ascontiguousarray(acc.T).reshape(1, SEQ, D_MODEL)



# revision 26
# speedup vs baseline: 1.1832x; 1.1832x over previous
"""Causal multi-head attention with RoPE on 8 Trainium2 NeuronCores.

Problem: x[1,4096,1024], 16 heads, head_dim 64, causal, RoPE(theta=1e4),
Q/K/V/O projections. Sharding: 2 heads per core (head-parallel). Each core:
  - computes Q^T,K^T (transposed, RoPE'd, bf16) and V (natural, bf16) for its
    2 heads
  - flash-style causal attention with scores kept transposed (S^T[k,q]) so
    P^T feeds the P@V matmul directly; softmax denominator comes from an
    appended ones-column in V (M=65 matmul); no max-subtraction needed
    (scores ~ N(0,1) -> exp never overflows)
  - o_proj partial (its 128 ctx columns x full Wo) -> out^T[1024,4096] bf16
Host: transposes x / weight slices (cast bf16), builds RoPE cos/sin tables
(f32), sums the 8 partial outputs (f32) and transposes back.

Perf notes (measured on HW, 327us baseline -> ~280us):
  - softmax 1/den: DVE Reciprocal is an 8-pass iterative divide (8 cyc per
    free elem) so a [1,1024] recip costs 6.8us; we DMA-bounce the dens to a
    [128,8] layout (SBUF->SBUF, linear walk), recip there (~200ns), DMA back.
  - o_part is written bf16 (halves output DMA; host sums in f32).
  - V transpose in bf16 via PE (1 cyc/row); psum evacuations are split
    between DVE and ScalarE by chunk phase (ScalarE is exp-saturated in late
    chunks, idle early; DVE gates the proj chain early).
  - K/V projection is prefetched one chunk ahead for early (PE-thin) chunks
    and computed just-in-time inside late (exp-bound) chunks.
  - dummy warm-keeper matmuls bracket the final tail so the HAM clock gate
    doesn't halve the PE clock during the last o_proj.
  - fp8 DoubleRow for P@V was tried and REVERTED: the rhs still streams one
    element/cycle in this layout (measured ~700ns vs 2x213ns bf16), and the
    exp->fp8 path NaN'd. reciprocal_approx_fast (custom DVE op) also NaN's
    on this NRT path - do not use.

Matmul operands are bf16 (1 cyc/row on PE); accumulation is f32 in PSUM.
"""
import os
import sys

sys.path.insert(0, "/opt/trn_rl_repo")

import numpy as np

D_MODEL = 1024
N_HEADS = 16
SEQ = 4096
DHEAD = 64
THETA = 10000.0
N_CORES = 8
CHUNK = 512          # seq chunk = q-block width
NKB = SEQ // 128     # 32 k-blocks of 128


def _build_program():
    from contextlib import ExitStack

    import concourse.bass as bass
    import concourse.mybir as mybir
    import concourse.tile as tile
    from concourse import bacc
    from concourse.masks import make_identity

    F32 = mybir.dt.float32
    F32R = mybir.dt.float32r
    BF16 = mybir.dt.bfloat16
    FP8 = mybir.dt.float8e4
    DR = mybir.MatmulPerfMode.DoubleRow
    AF = mybir.ActivationFunctionType

    nc = bacc.Bacc()

    xt_d = nc.dram_tensor("xt", [D_MODEL, SEQ], BF16, kind="ExternalInput")
    wq_d = nc.dram_tensor("wq", [D_MODEL, 128], BF16, kind="ExternalInput")
    wk_d = nc.dram_tensor("wk", [D_MODEL, 128], BF16, kind="ExternalInput")
    wv_d = nc.dram_tensor("wv", [D_MODEL, 128], BF16, kind="ExternalInput")
    wo_d = nc.dram_tensor("wo", [128, D_MODEL], BF16, kind="ExternalInput")
    t1_d = nc.dram_tensor("t1", [128, SEQ], F32, kind="ExternalInput")
    t2_d = nc.dram_tensor("t2", [128, SEQ], F32, kind="ExternalInput")
    m1_d = nc.dram_tensor("m1", [128, 128], BF16, kind="ExternalInput")
    m2_d = nc.dram_tensor("m2", [128, 128], BF16, kind="ExternalInput")
    o_d = nc.dram_tensor("o_part", [D_MODEL, SEQ], BF16, kind="ExternalOutput")

    NCHUNK = SEQ // CHUNK  # 8

    with tile.TileContext(nc) as tc:
        with nc.allow_low_precision(reason="bf16 compute; f32 accumulate"), \
             ExitStack() as ctx:
            const = ctx.enter_context(tc.tile_pool(name="const", bufs=1))
            persist = ctx.enter_context(tc.tile_pool(name="persist", bufs=1))
            work = ctx.enter_context(tc.tile_pool(name="work", bufs=1))
            psum = ctx.enter_context(tc.tile_pool(name="psum", bufs=1, space="PSUM"))

            m1_sb = const.tile([128, 128], BF16, name="m1_sb", tag="m1_sb")
            nc.sync.dma_start(m1_sb[:], m1_d[:])
            # PE warm-up: dense dummy matmuls during the DMA-bound head keep
            # the HAM clock-gate at full rate before real work arrives. m1 is
            # the first DMA issued so the PE lights up as early as possible.
            warm_ps = psum.tile([128, 128], F32, name="warm_ps", tag="misc",
                                bufs=2)
            for _ in range(48):
                nc.tensor.matmul(warm_ps[:], m1_sb[:], m1_sb[:],
                                 start=True, stop=True)
            ident = const.tile([128, 128], F32, name="ident", tag="ident")
            make_identity(nc, ident[:])
            identb = const.tile([128, 128], BF16, name="identb", tag="identb")
            nc.vector.tensor_copy(identb[:], ident[:])
            # ones2[h, :] selects head h's 64-row block: one matmul broadcasts
            # both heads' per-column 1/den [2,512] -> [128,512]. Built via a
            # linear SBUF->SBUF DMA (engines can't write at partition 1).
            o2flat = const.tile([1, 256], F32, name="o2flat", tag="o2flat")
            nc.vector.memset(o2flat[:], 0.0)
            nc.vector.memset(o2flat[:, 0:64], 1.0)
            nc.vector.memset(o2flat[:, 192:256], 1.0)
            o2r = const.tile([1, 256], F32R, name="o2r", tag="o2r")
            nc.vector.tensor_copy(o2r[:], o2flat[:])
            ones2 = const.tile([2, 128], F32R, name="ones2", tag="ones2")
            nc.sync.dma_start(ones2[:], o2r[:])
            onescol = const.tile([128, 2, 1], BF16, name="onescol", tag="onescol")
            nc.vector.memset(onescol[:], 1.0)
            zbias = const.tile([128, 1], F32, name="zbias", tag="zbias")
            nc.vector.memset(zbias[:], 0.0)

            m2_sb = const.tile([128, 128], BF16, name="m2_sb", tag="m2_sb")
            nc.sync.dma_start(m2_sb[:], m2_d[:])
            wo_sb = const.tile([128, D_MODEL], BF16, name="wo_sb", tag="wo_sb")

            wq_sb = const.tile([128, 8, 128], BF16, name="wq_sb", tag="wq_sb")
            wk_sb = const.tile([128, 8, 128], BF16, name="wk_sb", tag="wk_sb")
            wv_sb = const.tile([128, 8, 128], BF16, name="wv_sb", tag="wv_sb")
            w_sb = {"q": wq_sb, "k": wk_sb, "v": wv_sb}

            def load_w(d_t, sb):
                # d_t [1024, 128] viewed as [p 128, i 8, col 128]
                nc.sync.dma_start(
                    sb[:], d_t.rearrange("(i p) c -> p i c", i=8))

            load_w(wq_d, wq_sb)

            qt = [None] * NCHUNK   # Q^T chunks [128, 512] bf16 (RoPE'd, d-perm)
            kt = [None] * NCHUNK
            vsb = [None] * NKB     # V natural per k-block [128, 2, 65] bf16
            xts_c = {}
            xtsq_c = {}
            tabs_c = {}

            def load_chunk(c):
                cs = slice(c * CHUNK, (c + 1) * CHUNK)
                t = work.tile([128, 8, CHUNK], BF16, name=f"xt_{c}",
                              tag="xt", bufs=3)
                nc.sync.dma_start(
                    t[:], xt_d.rearrange("(i p) s -> p i s", i=8)[:, :, cs])
                xts_c[c] = t
                t1c = const.tile([128, CHUNK], F32, name=f"t1c{c}", tag=f"t1c{c}")
                nc.sync.dma_start(t1c[:], t1_d[:, cs])
                t2c = const.tile([128, CHUNK], F32, name=f"t2c{c}", tag=f"t2c{c}")
                nc.sync.dma_start(t2c[:], t2_d[:, cs])
                tabs_c[c] = (t1c, t2c)

            def proj(kind, c):
                w = w_sb[kind]
                ps = psum.tile([128, CHUNK], F32, name=f"{kind}ps{c}",
                               tag="misc", bufs=2)
                xts = xts_c[c]
                for i in range(8):
                    nc.tensor.matmul(ps[:], w[:, i, :], xts[:, i, :],
                                     start=(i == 0), stop=(i == 7))
                if kind in ("q", "k"):
                    t1c, t2c = tabs_c[c]
                    p1 = work.tile([128, CHUNK], BF16, name=f"p1_{kind}{c}",
                                   tag="p1", bufs=3)
                    nc.vector.tensor_mul(p1[:], t1c[:], ps[:])
                    p2 = work.tile([128, CHUNK], BF16, name=f"p2_{kind}{c}",
                                   tag="p2", bufs=3)
                    nc.vector.tensor_mul(p2[:], t2c[:], ps[:])
                    rp = psum.tile([128, CHUNK], F32, name=f"rp_{kind}{c}",
                                   tag="misc", bufs=2)
                    nc.tensor.matmul(rp[:], m1_sb[:], p1[:],
                                     start=True, stop=False)
                    nc.tensor.matmul(rp[:], m2_sb[:], p2[:],
                                     start=False, stop=True)
                    dst = persist.tile([128, CHUNK], BF16,
                                       name=f"{kind}t{c}", tag=f"{kind}t{c}")
                    # early chunks: ScalarE is idle and DVE gates the proj
                    # chain; late chunks: ScalarE is exp-saturated
                    if c <= 4:
                        nc.scalar.copy(dst[:], rp[:])
                    else:
                        nc.vector.tensor_copy(dst[:], rp[:])
                    if kind == "q":
                        qt[c] = dst
                    else:
                        kt[c] = dst
                else:
                    vt = work.tile([128, CHUNK], BF16, name=f"vt{c}",
                                   tag="vt", bufs=2)
                    if c <= 4:
                        nc.scalar.copy(vt[:], ps[:])
                    else:
                        nc.vector.tensor_copy(vt[:], ps[:])
                    vt_c[c] = vt

            vt_c = {}

            def vtrans(c, j):
                kb = c * 4 + j
                vn = psum.tile([128, 128], BF16, name=f"vn{kb}",
                               tag="misc", bufs=2)
                nc.tensor.transpose(vn[:],
                                    vt_c[c][:, j * 128:(j + 1) * 128],
                                    identb[:])
                vb = persist.tile([128, 2, 65], BF16, name=f"v{kb}",
                                  tag=f"v{kb}")
                if c <= 5:
                    nc.scalar.copy(vb[:, :, 0:64],
                                   vn[:].rearrange("p (h d) -> p h d", h=2))
                else:
                    nc.vector.tensor_copy(
                        vb[:, :, 0:64],
                        vn[:].rearrange("p (h d) -> p h d", h=2))
                nc.vector.tensor_copy(vb[:, :, 64:65], onescol[:])
                vsb[kb] = vb

            # deferred per-q-block state for the pipelined tail
            pend = {}
            pend_dsq = {}
            ctx_live = {}
            pend_ctx = {}   # qb -> list of (kbs, [ph_h0, ph_h1]) awaiting ctx MMs

            def attn_pair(qb, p0):
                # Scores for both heads of one k-block go into ONE psum tile
                # [128, 2, 512] (h-major columns): the h1 matmul shares the h0
                # matmul's buffer dependency, so the tile scheduler emits the
                # two K=64 tile_position-packed MMs back-to-back and they run
                # CONCURRENTLY on the PE (row groups 0-63 / 64-127). With the
                # old per-head tiles the h1 MM carried a semaphore wait on the
                # previous pair's h1 exp, which landed mid-stream of the h0 MM
                # and serialized the pair (trace: 317+216ns vs 317+4ns).
                # Diagonal blocks j=1,2 are column-trimmed (the first j*128
                # q-columns are fully masked); j=3 stays full so the stop=True
                # ctx matmul covers the whole accumulation region.
                nkb = 4 * (qb + 1)
                if qb not in ctx_live:
                    ctx_live[qb] = [
                        psum.tile([65, CHUNK], F32, name=f"ctx_{qb}_{h}",
                                  tag="ctx", bufs=2)
                        for h in (0, 1)]
                    pend_ctx[qb] = []
                kbs = list(range(p0, min(p0 + 2, nkb)))
                s2s = []
                for kb in kbs:
                    j = kb - 4 * qb
                    off = j * 128 if j in (1, 2, 3) else 0
                    s2 = psum.tile([128, 2, CHUNK], F32,
                                   name=f"s2_{qb}_{kb}", tag="scores",
                                   bufs=2)
                    for h in (0, 1):
                        nc.tensor.matmul(
                            s2[:, h, off:],
                            kt[kb // 4][h * 64:(h + 1) * 64,
                                        (kb % 4) * 128:(kb % 4) * 128 + 128],
                            qt[qb][h * 64:(h + 1) * 64, off:],
                            start=True, stop=True,
                            tile_position=(h * 64, 0))
                    s2s.append((kb, off, s2))
                entry = []
                for kb, off, s2 in s2s:
                    ph = work.tile([128, 2, CHUNK], BF16,
                                   name=f"ph_{qb}_{kb}", tag="p2h", bufs=8)
                    nc.scalar.activation(ph[:, :, off:], s2[:, :, off:],
                                         AF.Exp, bias=zbias[:], scale=0.125)
                    sl = []
                    if kb >= 4 * qb:  # diagonal: zero k_global > q_global
                        ncols = CHUNK - off
                        for h in (0, 1):
                            pm = work.tile([128, CHUNK], BF16,
                                           name=f"pm_{qb}_{kb}_{h}",
                                           tag="phm", bufs=6)
                            nc.gpsimd.affine_select(
                                out=pm[:, 0:ncols],
                                in_=ph[:, h, off:],
                                pattern=[[1, ncols]],
                                compare_op=mybir.AluOpType.is_ge,
                                fill=0.0,
                                base=-(kb - 4 * qb) * 128 + off,
                                channel_multiplier=-1)
                            sl.append((pm, None, 0, ncols, off))
                    else:
                        for h in (0, 1):
                            sl.append((ph, h, 0, CHUNK, 0))
                    entry.append((kb, sl))
                pend_ctx[qb].append(entry)
                # emit ctx for the PREVIOUS pending pair (depth-1 pipeline)
                if len(pend_ctx[qb]) > 1:
                    _emit_ctx_entry(qb, pend_ctx[qb].pop(0))

            def _emit_ctx_entry(qb, entry):
                nkb = 4 * (qb + 1)
                ctx_ps = ctx_live[qb]
                for h in (0, 1):
                    for kb, sl in entry:
                        tile_, hsel, lo, hi, ooff = sl[h]
                        rhs = (tile_[:, lo:hi] if hsel is None
                               else tile_[:, hsel, lo:hi])
                        nc.tensor.matmul(
                            ctx_ps[h][:, ooff:],
                            vsb[kb][:, h, :],
                            rhs,
                            start=(kb == 0), stop=(kb == nkb - 1),
                            skip_group_check=(ooff > 0))

            def attn_finish(qb):
                # flush remaining pending ctx pairs, evacuate ctx psum to SBUF
                # (frees the psum slots for the next q-block immediately), then
                # 1/den = exp(-ln(den)) on ScalarE (same act-table set as the
                # softmax exp), written bf16 for the broadcast matmul.
                for entry_prev in pend_ctx.pop(qb):
                    _emit_ctx_entry(qb, entry_prev)
                ctx_ps = ctx_live.pop(qb)
                ctxs = []
                den = work.tile([1, 2 * CHUNK], F32, name=f"den{qb}",
                                tag="den", bufs=2)
                # den rows first: starts the reciprocal DMA round-trip while
                # the (bigger) ctx evacuations still run; the two heads go to
                # different engines so the copies overlap
                nc.vector.tensor_copy(den[:, 0:CHUNK], ctx_ps[0][64:65, :])
                nc.scalar.copy(den[:, CHUNK:2 * CHUNK], ctx_ps[1][64:65, :])
                # the [1,1024] reciprocal is free-size bound (8 ALU passes
                # over the free dim); bounce via DMA to [128,8] where the
                # same op costs ~70ns, then DMA back. (Tried ScalarE
                # exp(-ln(x)) for the final chunk instead: walrus picks the
                # exp_and_others act table set, so the Ln costs TWO mid-tail
                # ACT_TABLE_LOADs (~2.6us) - a net loss. Do not repeat.)
                dsq = work.tile([128, 8], F32, name=f"dsq{qb}",
                                tag="dsq", bufs=2)
                nc.sync.dma_start(dsq[:], den[:])
                pend_dsq[qb] = dsq
                rsq = work.tile([128, 8], F32R, name=f"rsq{qb}",
                                tag="rsq", bufs=2)
                nc.vector.reciprocal(rsq[:], dsq[:])
                rec = work.tile([2, CHUNK], F32R, name=f"rec{qb}",
                                tag="rec", bufs=2)
                nc.sync.dma_start(rec[:], rsq[:])
                flat = False
                for h in (0, 1):
                    cs_ = work.tile([64, CHUNK], F32, name=f"ctxs{qb}{h}",
                                    tag="ctxs", bufs=4)
                    nc.vector.tensor_copy(cs_[:], ctx_ps[h][0:64, :])
                    ctxs.append(cs_)
                pend[qb] = (ctxs, rec, flat)

            ctxn_live = {}

            def tail_norm(qb):
                ctxs, rec, flat = pend.pop(qb)
                ctxn = work.tile([128, CHUNK], BF16, name=f"ctxn{qb}",
                                 tag="ctxn", bufs=2)
                bc = psum.tile([128, CHUNK], F32, name=f"bc{qb}",
                               tag="misc", bufs=2)
                if flat:
                    # rec is [1, 2*CHUNK]: broadcast via two accumulated K=1
                    # matmuls (o2r halves select each head's 64 rows)
                    nc.tensor.matmul(bc[:], o2r[:, 0:128], rec[:, 0:CHUNK],
                                     start=True, stop=False)
                    nc.tensor.matmul(bc[:], o2r[:, 128:256], rec[:, CHUNK:],
                                     start=False, stop=True)
                else:
                    nc.tensor.matmul(bc[:], ones2[:], rec[:],
                                     start=True, stop=True)
                for h in (0, 1):
                    nc.vector.tensor_mul(ctxn[h * 64:(h + 1) * 64, :],
                                         ctxs[h][:], bc[h * 64:(h + 1) * 64, :])
                ctxn_live[qb] = ctxn

            osb8_live = {}

            def get_osb8(qb):
                if qb not in osb8_live:
                    osb8_live[qb] = work.tile([128, 8, CHUNK], BF16,
                                              name=f"osb8_{qb}", tag="osb8",
                                              bufs=2)
                return osb8_live[qb]

            def tail_oproj(qb, obs, split=False):
                ctxn = ctxn_live[qb]
                if split:
                    # endgame: per-ob half-width MMs into separate banks so
                    # DVE and ScalarE evacuate in parallel, all into ONE
                    # tile, then just 2 big DMAs — the final tail was paying
                    # ~600ns of serial Sync-engine setup per small DMA.
                    H = CHUNK // 2
                    osb8 = get_osb8(qb)
                    for ob in obs:
                        wslice = wo_sb[:, ob * 128:(ob + 1) * 128]
                        pa = psum.tile([128, H], F32, name=f"opsa{qb}{ob}",
                                       tag="misc", bufs=2)
                        pb = psum.tile([128, H], F32, name=f"opsb{qb}{ob}",
                                       tag="misc", bufs=2)
                        nc.tensor.matmul(pa[:], wslice, ctxn[:, 0:H],
                                         start=True, stop=True)
                        nc.tensor.matmul(pb[:], wslice, ctxn[:, H:],
                                         start=True, stop=True)
                        nc.vector.tensor_copy(osb8[:, ob, 0:H], pa[:])
                        nc.scalar.copy(osb8[:, ob, H:], pb[:])
                        if ob == 3:
                            nc.sync.dma_start(
                                o_d[0:512, qb * CHUNK:(qb + 1) * CHUNK]
                                .rearrange("(ob p) s -> p ob s", ob=4),
                                osb8[:, 0:4, :])
                    nc.sync.dma_start(
                        o_d[512:1024, qb * CHUNK:(qb + 1) * CHUNK]
                        .rearrange("(ob p) s -> p ob s", ob=4),
                        osb8[:, 4:8, :])
                    return
                # batched: evacuate all obs into one [128,8,512] tile and
                # issue just TWO output DMAs per chunk (each dma_start costs
                # ~670ns of serial sync-engine setup; 8 small DMAs/chunk had
                # the sync sequencer 60% busy)
                osb8 = get_osb8(qb)
                for ob in obs:
                    wslice = wo_sb[:, ob * 128:(ob + 1) * 128]
                    o_ps = psum.tile([128, CHUNK], F32, name=f"ops{qb}{ob}",
                                     tag="misc", bufs=2)
                    nc.tensor.matmul(o_ps[:], wslice,
                                     ctxn[:], start=True, stop=True)
                    nc.vector.tensor_copy(osb8[:, ob, :], o_ps[:])
                    if ob == 3:
                        nc.sync.dma_start(
                            o_d[0:512, qb * CHUNK:(qb + 1) * CHUNK]
                            .rearrange("(ob p) s -> p ob s", ob=4),
                            osb8[:, 0:4, :])
                    elif ob == 7:
                        nc.sync.dma_start(
                            o_d[512:1024, qb * CHUNK:(qb + 1) * CHUNK]
                            .rearrange("(ob p) s -> p ob s", ob=4),
                            osb8[:, 4:8, :])

            # prologue: chunk 0's full QKV + chunk 1's q-proj. K/V of chunk
            # c>=1 is computed just-in-time DURING chunk c's early pairs (it
            # is first needed by the two diagonal pairs at the end) — this
            # moves PE work out of the PE-bound early chunks into the
            # exp-bound late chunks where the PE would otherwise idle and
            # re-throttle.
            load_chunk(0)
            proj("q", 0)
            load_w(wk_d, wk_sb)
            proj("k", 0)
            load_w(wv_d, wv_sb)
            load_chunk(1)
            proj("v", 0)
            for j in range(4):
                vtrans(0, j)
            proj("q", 1)
            # filler MMs after the prologue's real work: they plug the
            # RoPE-chain dependency stalls at the chunk 0/1 boundary so the
            # HAM clock gate never sees an idle window early on
            for _ in range(24):
                nc.tensor.matmul(warm_ps[:], m1_sb[:], m1_sb[:],
                                 start=True, stop=True)

            for c in range(NCHUNK):
                npair = 2 * (c + 1)
                # (fraction-of-chunk, unit); kv units carry a deadline slot
                U = []
                if c > 0:
                    U.append((0.0, None, lambda qb=c - 1: attn_finish(qb)))
                if c >= 3:
                    # mid/late chunks are exp-bound: compute own K/V just-in-
                    # time during the early pairs to fill otherwise-idle PE
                    kvcap = max(2 * c - 1, 1)
                    U.append((0.0, kvcap, lambda c=c: proj("k", c)))
                    U.append((0.08, kvcap, lambda c=c: proj("v", c)))
                    for j in range(4):
                        U.append((0.15 + 0.1 * j, kvcap,
                                  lambda c=c, j=j: vtrans(c, j)))
                if c <= 1:
                    # early chunks are PE-thin: prefetch next chunk's K/V
                    U.append((0.2, None, lambda c=c: proj("k", c + 1)))
                    U.append((0.3, None, lambda c=c: proj("v", c + 1)))
                    for j in range(4):
                        U.append((0.4 + 0.12 * j, None,
                                  lambda c=c, j=j: vtrans(c + 1, j)))
                if c == 1:
                    U.append((0.0, None,
                              lambda: nc.sync.dma_start(wo_sb[:], wo_d[:])))
                if c > 0:
                    U.append((0.35, None, lambda qb=c - 1: tail_norm(qb)))
                    U.append((0.5, None,
                              lambda qb=c - 1: tail_oproj(qb, range(0, 4))))
                    U.append((0.65, None,
                              lambda qb=c - 1: tail_oproj(qb, range(4, 8))))
                if 1 <= c <= 3:
                    # HAM feed: the early phase has PE-idle pockets (rope/DVE
                    # dependency stalls) long enough to re-throttle the clock
                    # (state-1 window +51..68us in the trace); LDWEIGHTS count
                    # as PE activity without touching PSUM
                    for frac in (0.3, 0.6, 0.9):
                        U.append((frac, None, lambda: (
                            nc.tensor.ldweights(weights=m1_sb[:]),
                            nc.tensor.ldweights(weights=m2_sb[:]))))
                if c < NCHUNK - 2:
                    U.append((0.4, None, lambda c=c: load_chunk(c + 2)))
                if c < NCHUNK - 1 and c > 0:
                    U.append((0.8, None, lambda c=c: proj("q", c + 1)))

                slots = {}
                for frac, cap, u in U:
                    s = int(frac * npair)
                    if cap is not None:
                        s = min(s, cap)
                    slots.setdefault(s, []).append(u)
                for p in range(npair):
                    attn_pair(c, 2 * p)
                    for u in slots.pop(p, []):
                        u()
                for rest in sorted(slots):
                    for u in slots[rest]:
                        u()
            attn_finish(NCHUNK - 1)
            # keep the PE warm through the final den DMA round-trip (no real
            # PE work exists there) so the last o_proj runs at full clock.
            # Pacer MMs READ the recip chain's intermediates so the warm
            # batches are spread across the whole round-trip instead of all
            # draining instantly at the start (which let HAM re-throttle and
            # the final o_proj ran at half clock, 348ns vs ~135ns per MM).
            warm2 = psum.tile([128, 128], F32, name="warm2", tag="scores",
                              bufs=2)
            dsq7 = pend_dsq[NCHUNK - 1]
            ctxs7, rec7, _flat7 = pend[NCHUNK - 1]
            # N=128 warm MMs on a scores-tag psum tile (misc-tag would contend
            # with the o_proj/bc ring); batch sizes match each chain stage's
            # latency so the HAM activity window never sees a >3.4us idle
            for _ in range(42):
                nc.tensor.matmul(warm2[:], m1_sb[:], m1_sb[:],
                                 start=True, stop=True)
            # pacer: runs only after the first bounce DMA lands
            nc.tensor.matmul(warm2[:, 0:8], ident[:], dsq7[:],
                             start=True, stop=True)
            for _ in range(26):
                nc.tensor.matmul(warm2[:], m1_sb[:], m1_sb[:],
                                 start=True, stop=True)
            # pacer: runs only after the second bounce DMA lands
            nc.tensor.matmul(warm2[:, 0:64], ones2[:], rec7[:, 0:64],
                             start=True, stop=True)
            for _ in range(4):
                nc.tensor.matmul(warm2[:], m1_sb[:], m1_sb[:],
                                 start=True, stop=True)
            tail_norm(NCHUNK - 1)
            # fill the bc->ctxn-mul latency so HAM stays warm into the o_proj
            for _ in range(10):
                nc.tensor.matmul(warm2[:], m1_sb[:], m1_sb[:],
                                 start=True, stop=True)
            tail_oproj(NCHUNK - 1, range(0, 8), split=True)

    nc.compile()
    return nc


_PROG = None


def _get_prog():
    global _PROG
    if _PROG is None:
        _PROG = _build_program()
    return _PROG


def _make_in_maps(inputs):
    import ml_dtypes
    bf16 = ml_dtypes.bfloat16
    x = np.asarray(inputs["x"], dtype=np.float32)
    Wq = np.asarray(inputs["Wq"], dtype=np.float32)
    Wk = np.asarray(inputs["Wk"], dtype=np.float32)
    Wv = np.asarray(inputs["Wv"], dtype=np.float32)
    Wo = np.asarray(inputs["Wo"], dtype=np.float32)
    pos = np.asarray(inputs["token_positions"]).astype(np.float32)

    xt = np.ascontiguousarray(x.reshape(SEQ, D_MODEL).T.astype(bf16))

    ks = np.arange(0, DHEAD, 2, dtype=np.float32)
    inv_freq = (1.0 / np.power(np.float32(THETA), ks / np.float32(DHEAD))).astype(np.float32)
    ang = pos[:, None] * inv_freq[None, :]          # [SEQ, 32]
    cosT = np.cos(ang).T.astype(np.float32)         # [32, SEQ]
    sinT = np.sin(ang).T.astype(np.float32)
    t1 = np.ascontiguousarray(np.concatenate([cosT, -sinT, cosT, -sinT], axis=0))
    t2 = np.ascontiguousarray(np.concatenate([sinT, cosT, sinT, cosT], axis=0))

    perm = np.concatenate([np.arange(0, DHEAD, 2), np.arange(1, DHEAD, 2)])

    m1 = np.zeros((128, 128), dtype=np.float32)
    m2 = np.zeros((128, 128), dtype=np.float32)
    for m in range(128):
        if m % 64 < 32:
            m1[m, m] = 1.0
            m1[m + 32, m] = 1.0
        else:
            m2[m - 32, m] = 1.0
            m2[m, m] = 1.0
    m1 = np.ascontiguousarray(m1.astype(bf16))
    m2 = np.ascontiguousarray(m2.astype(bf16))



    in_maps = []
    for c in range(N_CORES):
        rows = np.arange(c * 128, (c + 1) * 128)
        qk_rows = np.concatenate([c * 128 + h * DHEAD + perm for h in (0, 1)])
        in_maps.append({
            "xt": xt,
            "wq": np.ascontiguousarray(Wq[qk_rows, :].T.astype(bf16)),
            "wk": np.ascontiguousarray(Wk[qk_rows, :].T.astype(bf16)),
            "wv": np.ascontiguousarray(Wv[rows, :].T.astype(bf16)),
            "wo": np.ascontiguousarray(Wo[:, rows].T.astype(bf16)),
            "t1": t1,
            "t2": t2,
            "m1": m1,
            "m2": m2,
        })
    return in_maps


def kernel(x, Wq, Wk, Wv, Wo, token_positions):
    nc = _get_prog()
    in_maps = _make_in_maps({"x": x, "Wq": Wq, "Wk": Wk, "Wv": Wv, "Wo": Wo,
                             "token_positions": token_positions})
    from concourse.bass_utils import run_bass_kernel_spmd

    res = run_bass_kernel_spmd(nc, in_maps, core_ids=list(range(N_CORES)))
    acc = res.results[0]["o_part"].astype(np.float32)
    for i in range(1, N_CORES):
        acc = acc + res.results[i]["o_part"].astype(np.float32)
    return np.ascontiguousarray(acc.T).reshape(1, SEQ, D_MODEL)



# revision 27
# speedup vs baseline: 1.1943x; 1.0094x over previous
"""Causal multi-head attention with RoPE on 8 Trainium2 NeuronCores.

Problem: x[1,4096,1024], 16 heads, head_dim 64, causal, RoPE(theta=1e4),
Q/K/V/O projections. Sharding: 2 heads per core (head-parallel). Each core:
  - computes Q^T,K^T (transposed, RoPE'd, bf16) and V (natural, bf16) for its
    2 heads
  - flash-style causal attention with scores kept transposed (S^T[k,q]) so
    P^T feeds the P@V matmul directly; softmax denominator comes from an
    appended ones-column in V (M=65 matmul); no max-subtraction needed
    (scores ~ N(0,1) -> exp never overflows)
  - o_proj partial (its 128 ctx columns x full Wo) -> out^T[1024,4096] bf16
Host: transposes x / weight slices (cast bf16), builds RoPE cos/sin tables
(f32), sums the 8 partial outputs (f32) and transposes back.

Perf notes (measured on HW, 327us baseline -> ~280us):
  - softmax 1/den: DVE Reciprocal is an 8-pass iterative divide (8 cyc per
    free elem) so a [1,1024] recip costs 6.8us; we DMA-bounce the dens to a
    [128,8] layout (SBUF->SBUF, linear walk), recip there (~200ns), DMA back.
  - o_part is written bf16 (halves output DMA; host sums in f32).
  - V transpose in bf16 via PE (1 cyc/row); psum evacuations are split
    between DVE and ScalarE by chunk phase (ScalarE is exp-saturated in late
    chunks, idle early; DVE gates the proj chain early).
  - K/V projection is prefetched one chunk ahead for early (PE-thin) chunks
    and computed just-in-time inside late (exp-bound) chunks.
  - dummy warm-keeper matmuls bracket the final tail so the HAM clock gate
    doesn't halve the PE clock during the last o_proj.
  - fp8 DoubleRow for P@V was tried and REVERTED: the rhs still streams one
    element/cycle in this layout (measured ~700ns vs 2x213ns bf16), and the
    exp->fp8 path NaN'd. reciprocal_approx_fast (custom DVE op) also NaN's
    on this NRT path - do not use.

Matmul operands are bf16 (1 cyc/row on PE); accumulation is f32 in PSUM.
"""
import os
import sys

sys.path.insert(0, "/opt/trn_rl_repo")
# the tunneled device occasionally lands in a degraded (post-error) power
# state that slows every matmul stream ~20%; a core reset at init restores
# the full clock and costs no HW exec time
os.environ.setdefault("NEURON_RT_RESET_CORES", "1")

import numpy as np

D_MODEL = 1024
N_HEADS = 16
SEQ = 4096
DHEAD = 64
THETA = 10000.0
N_CORES = 8
CHUNK = 512          # seq chunk = q-block width
NKB = SEQ // 128     # 32 k-blocks of 128


def _build_program():
    from contextlib import ExitStack

    import concourse.bass as bass
    import concourse.mybir as mybir
    import concourse.tile as tile
    from concourse import bacc
    from concourse.masks import make_identity

    F32 = mybir.dt.float32
    F32R = mybir.dt.float32r
    BF16 = mybir.dt.bfloat16
    FP8 = mybir.dt.float8e4
    DR = mybir.MatmulPerfMode.DoubleRow
    AF = mybir.ActivationFunctionType

    nc = bacc.Bacc()

    xt_d = nc.dram_tensor("xt", [D_MODEL, SEQ], BF16, kind="ExternalInput")
    wq_d = nc.dram_tensor("wq", [D_MODEL, 128], BF16, kind="ExternalInput")
    wk_d = nc.dram_tensor("wk", [D_MODEL, 128], BF16, kind="ExternalInput")
    wv_d = nc.dram_tensor("wv", [D_MODEL, 128], BF16, kind="ExternalInput")
    wo_d = nc.dram_tensor("wo", [128, D_MODEL], BF16, kind="ExternalInput")
    t1_d = nc.dram_tensor("t1", [128, SEQ], F32, kind="ExternalInput")
    t2_d = nc.dram_tensor("t2", [128, SEQ], F32, kind="ExternalInput")
    m1_d = nc.dram_tensor("m1", [128, 128], BF16, kind="ExternalInput")
    m2_d = nc.dram_tensor("m2", [128, 128], BF16, kind="ExternalInput")
    o_d = nc.dram_tensor("o_part", [D_MODEL, SEQ], BF16, kind="ExternalOutput")

    NCHUNK = SEQ // CHUNK  # 8

    with tile.TileContext(nc) as tc:
        with nc.allow_low_precision(reason="bf16 compute; f32 accumulate"), \
             ExitStack() as ctx:
            const = ctx.enter_context(tc.tile_pool(name="const", bufs=1))
            persist = ctx.enter_context(tc.tile_pool(name="persist", bufs=1))
            work = ctx.enter_context(tc.tile_pool(name="work", bufs=1))
            psum = ctx.enter_context(tc.tile_pool(name="psum", bufs=1, space="PSUM"))

            m1_sb = const.tile([128, 128], BF16, name="m1_sb", tag="m1_sb")
            nc.sync.dma_start(m1_sb[:], m1_d[:])
            # PE warm-up: dense dummy matmuls during the DMA-bound head keep
            # the HAM clock-gate at full rate before real work arrives. m1 is
            # the first DMA issued so the PE lights up as early as possible.
            warm_ps = psum.tile([128, 128], F32, name="warm_ps", tag="misc",
                                bufs=2)
            for _ in range(48):
                nc.tensor.matmul(warm_ps[:], m1_sb[:], m1_sb[:],
                                 start=True, stop=True)
            ident = const.tile([128, 128], F32, name="ident", tag="ident")
            make_identity(nc, ident[:])
            identb = const.tile([128, 128], BF16, name="identb", tag="identb")
            nc.vector.tensor_copy(identb[:], ident[:])
            # ones2[h, :] selects head h's 64-row block: one matmul broadcasts
            # both heads' per-column 1/den [2,512] -> [128,512]. Built via a
            # linear SBUF->SBUF DMA (engines can't write at partition 1).
            o2flat = const.tile([1, 256], F32, name="o2flat", tag="o2flat")
            nc.vector.memset(o2flat[:], 0.0)
            nc.vector.memset(o2flat[:, 0:64], 1.0)
            nc.vector.memset(o2flat[:, 192:256], 1.0)
            o2r = const.tile([1, 256], F32R, name="o2r", tag="o2r")
            nc.vector.tensor_copy(o2r[:], o2flat[:])
            ones2 = const.tile([2, 128], F32R, name="ones2", tag="ones2")
            nc.sync.dma_start(ones2[:], o2r[:])
            onescol = const.tile([128, 2, 1], BF16, name="onescol", tag="onescol")
            nc.vector.memset(onescol[:], 1.0)
            zbias = const.tile([128, 1], F32, name="zbias", tag="zbias")
            nc.vector.memset(zbias[:], 0.0)

            m2_sb = const.tile([128, 128], BF16, name="m2_sb", tag="m2_sb")
            nc.sync.dma_start(m2_sb[:], m2_d[:])
            wo_sb = const.tile([128, D_MODEL], BF16, name="wo_sb", tag="wo_sb")

            wq_sb = const.tile([128, 8, 128], BF16, name="wq_sb", tag="wq_sb")
            wk_sb = const.tile([128, 8, 128], BF16, name="wk_sb", tag="wk_sb")
            wv_sb = const.tile([128, 8, 128], BF16, name="wv_sb", tag="wv_sb")
            w_sb = {"q": wq_sb, "k": wk_sb, "v": wv_sb}

            def load_w(d_t, sb):
                # d_t [1024, 128] viewed as [p 128, i 8, col 128]
                nc.sync.dma_start(
                    sb[:], d_t.rearrange("(i p) c -> p i c", i=8))

            load_w(wq_d, wq_sb)

            qt = [None] * NCHUNK   # Q^T chunks [128, 512] bf16 (RoPE'd, d-perm)
            kt = [None] * NCHUNK
            vsb = [None] * NKB     # V natural per k-block [128, 2, 65] bf16
            xts_c = {}
            xtsq_c = {}
            tabs_c = {}

            def load_chunk(c):
                cs = slice(c * CHUNK, (c + 1) * CHUNK)
                t = work.tile([128, 8, CHUNK], BF16, name=f"xt_{c}",
                              tag="xt", bufs=3)
                nc.sync.dma_start(
                    t[:], xt_d.rearrange("(i p) s -> p i s", i=8)[:, :, cs])
                xts_c[c] = t
                t1c = const.tile([128, CHUNK], F32, name=f"t1c{c}", tag=f"t1c{c}")
                nc.sync.dma_start(t1c[:], t1_d[:, cs])
                t2c = const.tile([128, CHUNK], F32, name=f"t2c{c}", tag=f"t2c{c}")
                nc.sync.dma_start(t2c[:], t2_d[:, cs])
                tabs_c[c] = (t1c, t2c)

            def proj(kind, c):
                w = w_sb[kind]
                ps = psum.tile([128, CHUNK], F32, name=f"{kind}ps{c}",
                               tag="misc", bufs=2)
                xts = xts_c[c]
                for i in range(8):
                    nc.tensor.matmul(ps[:], w[:, i, :], xts[:, i, :],
                                     start=(i == 0), stop=(i == 7))
                if kind in ("q", "k"):
                    t1c, t2c = tabs_c[c]
                    p1 = work.tile([128, CHUNK], BF16, name=f"p1_{kind}{c}",
                                   tag="p1", bufs=3)
                    nc.vector.tensor_mul(p1[:], t1c[:], ps[:])
                    p2 = work.tile([128, CHUNK], BF16, name=f"p2_{kind}{c}",
                                   tag="p2", bufs=3)
                    nc.vector.tensor_mul(p2[:], t2c[:], ps[:])
                    rp = psum.tile([128, CHUNK], F32, name=f"rp_{kind}{c}",
                                   tag="misc", bufs=2)
                    nc.tensor.matmul(rp[:], m1_sb[:], p1[:],
                                     start=True, stop=False)
                    nc.tensor.matmul(rp[:], m2_sb[:], p2[:],
                                     start=False, stop=True)
                    dst = persist.tile([128, CHUNK], BF16,
                                       name=f"{kind}t{c}", tag=f"{kind}t{c}")
                    # early chunks: ScalarE is idle and DVE gates the proj
                    # chain; late chunks: ScalarE is exp-saturated
                    if c <= 4:
                        nc.scalar.copy(dst[:], rp[:])
                    else:
                        nc.vector.tensor_copy(dst[:], rp[:])
                    if kind == "q":
                        qt[c] = dst
                    else:
                        kt[c] = dst
                else:
                    vt = work.tile([128, CHUNK], BF16, name=f"vt{c}",
                                   tag="vt", bufs=2)
                    if c <= 4:
                        nc.scalar.copy(vt[:], ps[:])
                    else:
                        nc.vector.tensor_copy(vt[:], ps[:])
                    vt_c[c] = vt

            vt_c = {}

            def vtrans(c, j):
                kb = c * 4 + j
                vn = psum.tile([128, 128], BF16, name=f"vn{kb}",
                               tag="misc", bufs=2)
                nc.tensor.transpose(vn[:],
                                    vt_c[c][:, j * 128:(j + 1) * 128],
                                    identb[:])
                vb = persist.tile([128, 2, 65], BF16, name=f"v{kb}",
                                  tag=f"v{kb}")
                if c <= 5:
                    nc.scalar.copy(vb[:, :, 0:64],
                                   vn[:].rearrange("p (h d) -> p h d", h=2))
                else:
                    nc.vector.tensor_copy(
                        vb[:, :, 0:64],
                        vn[:].rearrange("p (h d) -> p h d", h=2))
                nc.vector.tensor_copy(vb[:, :, 64:65], onescol[:])
                vsb[kb] = vb

            # deferred per-q-block state for the pipelined tail
            pend = {}
            pend_dsq = {}
            ctx_live = {}
            pend_ctx = {}   # qb -> list of (kbs, [ph_h0, ph_h1]) awaiting ctx MMs

            def attn_pair(qb, p0):
                # Scores for both heads of one k-block go into ONE psum tile
                # [128, 2, 512] (h-major columns): the h1 matmul shares the h0
                # matmul's buffer dependency, so the tile scheduler emits the
                # two K=64 tile_position-packed MMs back-to-back and they run
                # CONCURRENTLY on the PE (row groups 0-63 / 64-127). With the
                # old per-head tiles the h1 MM carried a semaphore wait on the
                # previous pair's h1 exp, which landed mid-stream of the h0 MM
                # and serialized the pair (trace: 317+216ns vs 317+4ns).
                # Diagonal blocks j=1,2 are column-trimmed (the first j*128
                # q-columns are fully masked); j=3 stays full so the stop=True
                # ctx matmul covers the whole accumulation region.
                nkb = 4 * (qb + 1)
                if qb not in ctx_live:
                    ctx_live[qb] = [
                        psum.tile([65, CHUNK], F32, name=f"ctx_{qb}_{h}",
                                  tag="ctx", bufs=2)
                        for h in (0, 1)]
                    pend_ctx[qb] = []
                kbs = list(range(p0, min(p0 + 2, nkb)))
                s2s = []
                for kb in kbs:
                    j = kb - 4 * qb
                    off = j * 128 if j in (1, 2, 3) else 0
                    s2 = psum.tile([128, 2, CHUNK], F32,
                                   name=f"s2_{qb}_{kb}", tag="scores",
                                   bufs=2)
                    for h in (0, 1):
                        nc.tensor.matmul(
                            s2[:, h, off:],
                            kt[kb // 4][h * 64:(h + 1) * 64,
                                        (kb % 4) * 128:(kb % 4) * 128 + 128],
                            qt[qb][h * 64:(h + 1) * 64, off:],
                            start=True, stop=True,
                            tile_position=(h * 64, 0))
                    s2s.append((kb, off, s2))
                entry = []
                for kb, off, s2 in s2s:
                    ph = work.tile([128, 2, CHUNK], BF16,
                                   name=f"ph_{qb}_{kb}", tag="p2h", bufs=8)
                    nc.scalar.activation(ph[:, :, off:], s2[:, :, off:],
                                         AF.Exp, bias=zbias[:], scale=0.125)
                    sl = []
                    if kb >= 4 * qb:  # diagonal: zero k_global > q_global
                        ncols = CHUNK - off
                        for h in (0, 1):
                            pm = work.tile([128, CHUNK], BF16,
                                           name=f"pm_{qb}_{kb}_{h}",
                                           tag="phm", bufs=6)
                            nc.gpsimd.affine_select(
                                out=pm[:, 0:ncols],
                                in_=ph[:, h, off:],
                                pattern=[[1, ncols]],
                                compare_op=mybir.AluOpType.is_ge,
                                fill=0.0,
                                base=-(kb - 4 * qb) * 128 + off,
                                channel_multiplier=-1)
                            sl.append((pm, None, 0, ncols, off))
                    else:
                        for h in (0, 1):
                            sl.append((ph, h, 0, CHUNK, 0))
                    entry.append((kb, sl))
                pend_ctx[qb].append(entry)
                # emit ctx for the PREVIOUS pending pair (depth-1 pipeline)
                if len(pend_ctx[qb]) > 1:
                    _emit_ctx_entry(qb, pend_ctx[qb].pop(0))

            def _emit_ctx_entry(qb, entry):
                nkb = 4 * (qb + 1)
                ctx_ps = ctx_live[qb]
                for h in (0, 1):
                    for kb, sl in entry:
                        tile_, hsel, lo, hi, ooff = sl[h]
                        rhs = (tile_[:, lo:hi] if hsel is None
                               else tile_[:, hsel, lo:hi])
                        nc.tensor.matmul(
                            ctx_ps[h][:, ooff:],
                            vsb[kb][:, h, :],
                            rhs,
                            start=(kb == 0), stop=(kb == nkb - 1),
                            skip_group_check=(ooff > 0))

            def attn_finish(qb):
                # flush remaining pending ctx pairs, evacuate ctx psum to SBUF
                # (frees the psum slots for the next q-block immediately), then
                # 1/den = exp(-ln(den)) on ScalarE (same act-table set as the
                # softmax exp), written bf16 for the broadcast matmul.
                for entry_prev in pend_ctx.pop(qb):
                    _emit_ctx_entry(qb, entry_prev)
                ctx_ps = ctx_live.pop(qb)
                ctxs = []
                den = work.tile([1, 2 * CHUNK], F32, name=f"den{qb}",
                                tag="den", bufs=2)
                # den rows first: starts the reciprocal DMA round-trip while
                # the (bigger) ctx evacuations still run; the two heads go to
                # different engines so the copies overlap
                nc.vector.tensor_copy(den[:, 0:CHUNK], ctx_ps[0][64:65, :])
                nc.scalar.copy(den[:, CHUNK:2 * CHUNK], ctx_ps[1][64:65, :])
                # the [1,1024] reciprocal is free-size bound (8 ALU passes
                # over the free dim); bounce via DMA to [128,8] where the
                # same op costs ~70ns, then DMA back. (Tried ScalarE
                # exp(-ln(x)) for the final chunk instead: walrus picks the
                # exp_and_others act table set, so the Ln costs TWO mid-tail
                # ACT_TABLE_LOADs (~2.6us) - a net loss. Do not repeat.)
                dsq = work.tile([128, 8], F32, name=f"dsq{qb}",
                                tag="dsq", bufs=2)
                nc.sync.dma_start(dsq[:], den[:])
                pend_dsq[qb] = dsq
                rsq = work.tile([128, 8], F32R, name=f"rsq{qb}",
                                tag="rsq", bufs=2)
                nc.vector.reciprocal(rsq[:], dsq[:])
                rec = work.tile([2, CHUNK], F32R, name=f"rec{qb}",
                                tag="rec", bufs=2)
                nc.sync.dma_start(rec[:], rsq[:])
                flat = False
                for h in (0, 1):
                    cs_ = work.tile([64, CHUNK], F32, name=f"ctxs{qb}{h}",
                                    tag="ctxs", bufs=4)
                    nc.vector.tensor_copy(cs_[:], ctx_ps[h][0:64, :])
                    ctxs.append(cs_)
                pend[qb] = (ctxs, rec, flat)

            ctxn_live = {}

            def tail_norm(qb):
                ctxs, rec, flat = pend.pop(qb)
                ctxn = work.tile([128, CHUNK], BF16, name=f"ctxn{qb}",
                                 tag="ctxn", bufs=2)
                bc = psum.tile([128, CHUNK], F32, name=f"bc{qb}",
                               tag="misc", bufs=2)
                if flat:
                    # rec is [1, 2*CHUNK]: broadcast via two accumulated K=1
                    # matmuls (o2r halves select each head's 64 rows)
                    nc.tensor.matmul(bc[:], o2r[:, 0:128], rec[:, 0:CHUNK],
                                     start=True, stop=False)
                    nc.tensor.matmul(bc[:], o2r[:, 128:256], rec[:, CHUNK:],
                                     start=False, stop=True)
                else:
                    nc.tensor.matmul(bc[:], ones2[:], rec[:],
                                     start=True, stop=True)
                for h in (0, 1):
                    nc.vector.tensor_mul(ctxn[h * 64:(h + 1) * 64, :],
                                         ctxs[h][:], bc[h * 64:(h + 1) * 64, :])
                ctxn_live[qb] = ctxn

            osb8_live = {}

            def get_osb8(qb):
                if qb not in osb8_live:
                    osb8_live[qb] = work.tile([128, 8, CHUNK], BF16,
                                              name=f"osb8_{qb}", tag="osb8",
                                              bufs=2)
                return osb8_live[qb]

            def tail_oproj(qb, obs, split=False):
                ctxn = ctxn_live[qb]
                if split:
                    # endgame: per-ob half-width MMs into separate banks so
                    # DVE and ScalarE evacuate in parallel, all into ONE
                    # tile, then just 2 big DMAs — the final tail was paying
                    # ~600ns of serial Sync-engine setup per small DMA.
                    H = CHUNK // 2
                    osb8 = get_osb8(qb)
                    for ob in obs:
                        wslice = wo_sb[:, ob * 128:(ob + 1) * 128]
                        pa = psum.tile([128, H], F32, name=f"opsa{qb}{ob}",
                                       tag="misc", bufs=2)
                        pb = psum.tile([128, H], F32, name=f"opsb{qb}{ob}",
                                       tag="misc", bufs=2)
                        nc.tensor.matmul(pa[:], wslice, ctxn[:, 0:H],
                                         start=True, stop=True)
                        nc.tensor.matmul(pb[:], wslice, ctxn[:, H:],
                                         start=True, stop=True)
                        nc.vector.tensor_copy(osb8[:, ob, 0:H], pa[:])
                        nc.scalar.copy(osb8[:, ob, H:], pb[:])
                        if ob == 3:
                            nc.sync.dma_start(
                                o_d[0:512, qb * CHUNK:(qb + 1) * CHUNK]
                                .rearrange("(ob p) s -> p ob s", ob=4),
                                osb8[:, 0:4, :])
                    nc.sync.dma_start(
                        o_d[512:1024, qb * CHUNK:(qb + 1) * CHUNK]
                        .rearrange("(ob p) s -> p ob s", ob=4),
                        osb8[:, 4:8, :])
                    return
                # batched: evacuate all obs into one [128,8,512] tile and
                # issue just TWO output DMAs per chunk (each dma_start costs
                # ~670ns of serial sync-engine setup; 8 small DMAs/chunk had
                # the sync sequencer 60% busy)
                osb8 = get_osb8(qb)
                for ob in obs:
                    wslice = wo_sb[:, ob * 128:(ob + 1) * 128]
                    o_ps = psum.tile([128, CHUNK], F32, name=f"ops{qb}{ob}",
                                     tag="misc", bufs=2)
                    nc.tensor.matmul(o_ps[:], wslice,
                                     ctxn[:], start=True, stop=True)
                    # final chunk: ScalarE is idle, so alternating engines
                    # doubles evac throughput (the 2-slot misc ring makes the
                    # o_proj evac-limited); mid-kernel ScalarE runs exps
                    if qb == NCHUNK - 1 and ob % 2 == 1:
                        nc.scalar.copy(osb8[:, ob, :], o_ps[:])
                    else:
                        nc.vector.tensor_copy(osb8[:, ob, :], o_ps[:])
                    if ob == 3:
                        nc.sync.dma_start(
                            o_d[0:512, qb * CHUNK:(qb + 1) * CHUNK]
                            .rearrange("(ob p) s -> p ob s", ob=4),
                            osb8[:, 0:4, :])
                    elif ob == 7:
                        nc.sync.dma_start(
                            o_d[512:1024, qb * CHUNK:(qb + 1) * CHUNK]
                            .rearrange("(ob p) s -> p ob s", ob=4),
                            osb8[:, 4:8, :])

            # prologue: chunk 0's full QKV + chunk 1's q-proj. K/V of chunk
            # c>=1 is computed just-in-time DURING chunk c's early pairs (it
            # is first needed by the two diagonal pairs at the end) — this
            # moves PE work out of the PE-bound early chunks into the
            # exp-bound late chunks where the PE would otherwise idle and
            # re-throttle.
            load_chunk(0)
            proj("q", 0)
            load_w(wk_d, wk_sb)
            proj("k", 0)
            load_w(wv_d, wv_sb)
            load_chunk(1)
            proj("v", 0)
            for j in range(4):
                vtrans(0, j)
            proj("q", 1)
            # filler MMs after the prologue's real work: they plug the
            # RoPE-chain dependency stalls at the chunk 0/1 boundary so the
            # HAM clock gate never sees an idle window early on
            for _ in range(24):
                nc.tensor.matmul(warm_ps[:], m1_sb[:], m1_sb[:],
                                 start=True, stop=True)

            for c in range(NCHUNK):
                npair = 2 * (c + 1)
                # (fraction-of-chunk, unit); kv units carry a deadline slot
                U = []
                if c > 0:
                    U.append((0.0, None, lambda qb=c - 1: attn_finish(qb)))
                if c >= 3:
                    # mid/late chunks are exp-bound: compute own K/V just-in-
                    # time during the early pairs to fill otherwise-idle PE
                    kvcap = max(2 * c - 1, 1)
                    U.append((0.0, kvcap, lambda c=c: proj("k", c)))
                    U.append((0.08, kvcap, lambda c=c: proj("v", c)))
                    for j in range(4):
                        U.append((0.15 + 0.1 * j, kvcap,
                                  lambda c=c, j=j: vtrans(c, j)))
                if c <= 1:
                    # early chunks are PE-thin: prefetch next chunk's K/V
                    U.append((0.2, None, lambda c=c: proj("k", c + 1)))
                    U.append((0.3, None, lambda c=c: proj("v", c + 1)))
                    for j in range(4):
                        U.append((0.4 + 0.12 * j, None,
                                  lambda c=c, j=j: vtrans(c + 1, j)))
                if c == 1:
                    U.append((0.0, None,
                              lambda: nc.sync.dma_start(wo_sb[:], wo_d[:])))
                if c > 0:
                    U.append((0.35, None, lambda qb=c - 1: tail_norm(qb)))
                    U.append((0.5, None,
                              lambda qb=c - 1: tail_oproj(qb, range(0, 4))))
                    U.append((0.65, None,
                              lambda qb=c - 1: tail_oproj(qb, range(4, 8))))
                if 1 <= c <= 3:
                    # HAM feed: the early phase has PE-idle pockets (rope/DVE
                    # dependency stalls) long enough to re-throttle the clock
                    # (state-1 window +51..68us in the trace); LDWEIGHTS count
                    # as PE activity without touching PSUM
                    for frac in (0.3, 0.6, 0.9):
                        U.append((frac, None, lambda: (
                            nc.tensor.ldweights(weights=m1_sb[:]),
                            nc.tensor.ldweights(weights=m2_sb[:]))))
                if c < NCHUNK - 2:
                    U.append((0.4, None, lambda c=c: load_chunk(c + 2)))
                if c < NCHUNK - 1 and c > 0:
                    U.append((0.8, None, lambda c=c: proj("q", c + 1)))

                slots = {}
                for frac, cap, u in U:
                    s = int(frac * npair)
                    if cap is not None:
                        s = min(s, cap)
                    slots.setdefault(s, []).append(u)
                for p in range(npair):
                    attn_pair(c, 2 * p)
                    for u in slots.pop(p, []):
                        u()
                for rest in sorted(slots):
                    for u in slots[rest]:
                        u()
            attn_finish(NCHUNK - 1)
            # keep the PE warm through the final den DMA round-trip (no real
            # PE work exists there) so the last o_proj runs at full clock.
            # Pacer MMs READ the recip chain's intermediates so the warm
            # batches are spread across the whole round-trip instead of all
            # draining instantly at the start (which let HAM re-throttle and
            # the final o_proj ran at half clock, 348ns vs ~135ns per MM).
            warm2 = psum.tile([128, 128], F32, name="warm2", tag="scores",
                              bufs=2)
            dsq7 = pend_dsq[NCHUNK - 1]
            ctxs7, rec7, _flat7 = pend[NCHUNK - 1]
            # N=128 warm MMs on a scores-tag psum tile (misc-tag would contend
            # with the o_proj/bc ring); batch sizes match each chain stage's
            # latency so the HAM activity window never sees a >3.4us idle
            for _ in range(42):
                nc.tensor.matmul(warm2[:], m1_sb[:], m1_sb[:],
                                 start=True, stop=True)
            # pacer: runs only after the first bounce DMA lands
            nc.tensor.matmul(warm2[:, 0:8], ident[:], dsq7[:],
                             start=True, stop=True)
            for _ in range(26):
                nc.tensor.matmul(warm2[:], m1_sb[:], m1_sb[:],
                                 start=True, stop=True)
            # pacer: runs only after the second bounce DMA lands
            nc.tensor.matmul(warm2[:, 0:64], ones2[:], rec7[:, 0:64],
                             start=True, stop=True)
            for _ in range(4):
                nc.tensor.matmul(warm2[:], m1_sb[:], m1_sb[:],
                                 start=True, stop=True)
            tail_norm(NCHUNK - 1)
            # fill the bc->ctxn-mul latency so HAM stays warm into the o_proj
            for _ in range(10):
                nc.tensor.matmul(warm2[:], m1_sb[:], m1_sb[:],
                                 start=True, stop=True)
            tail_oproj(NCHUNK - 1, range(0, 8))

    nc.compile()
    return nc


_PROG = None


def _get_prog():
    global _PROG
    if _PROG is None:
        _PROG = _build_program()
    return _PROG


def _make_in_maps(inputs):
    import ml_dtypes
    bf16 = ml_dtypes.bfloat16
    x = np.asarray(inputs["x"], dtype=np.float32)
    Wq = np.asarray(inputs["Wq"], dtype=np.float32)
    Wk = np.asarray(inputs["Wk"], dtype=np.float32)
    Wv = np.asarray(inputs["Wv"], dtype=np.float32)
    Wo = np.asarray(inputs["Wo"], dtype=np.float32)
    pos = np.asarray(inputs["token_positions"]).astype(np.float32)

    xt = np.ascontiguousarray(x.reshape(SEQ, D_MODEL).T.astype(bf16))

    ks = np.arange(0, DHEAD, 2, dtype=np.float32)
    inv_freq = (1.0 / np.power(np.float32(THETA), ks / np.float32(DHEAD))).astype(np.float32)
    ang = pos[:, None] * inv_freq[None, :]          # [SEQ, 32]
    cosT = np.cos(ang).T.astype(np.float32)         # [32, SEQ]
    sinT = np.sin(ang).T.astype(np.float32)
    t1 = np.ascontiguousarray(np.concatenate([cosT, -sinT, cosT, -sinT], axis=0))
    t2 = np.ascontiguousarray(np.concatenate([sinT, cosT, sinT, cosT], axis=0))

    perm = np.concatenate([np.arange(0, DHEAD, 2), np.arange(1, DHEAD, 2)])

    m1 = np.zeros((128, 128), dtype=np.float32)
    m2 = np.zeros((128, 128), dtype=np.float32)
    for m in range(128):
        if m % 64 < 32:
            m1[m, m] = 1.0
            m1[m + 32, m] = 1.0
        else:
            m2[m - 32, m] = 1.0
            m2[m, m] = 1.0
    m1 = np.ascontiguousarray(m1.astype(bf16))
    m2 = np.ascontiguousarray(m2.astype(bf16))



    in_maps = []
    for c in range(N_CORES):
        rows = np.arange(c * 128, (c + 1) * 128)
        qk_rows = np.concatenate([c * 128 + h * DHEAD + perm for h in (0, 1)])
        in_maps.append({
            "xt": xt,
            "wq": np.ascontiguousarray(Wq[qk_rows, :].T.astype(bf16)),
            "wk": np.ascontiguousarray(Wk[qk_rows, :].T.astype(bf16)),
            "wv": np.ascontiguousarray(Wv[rows, :].T.astype(bf16)),
            "wo": np.ascontiguousarray(Wo[:, rows].T.astype(bf16)),
            "t1": t1,
            "t2": t2,
            "m1": m1,
            "m2": m2,
        })
    return in_maps


def kernel(x, Wq, Wk, Wv, Wo, token_positions):
    nc = _get_prog()
    in_maps = _make_in_maps({"x": x, "Wq": Wq, "Wk": Wk, "Wv": Wv, "Wo": Wo,
                             "token_positions": token_positions})
    from concourse.bass_utils import run_bass_kernel_spmd

    res = run_bass_kernel_spmd(nc, in_maps, core_ids=list(range(N_CORES)))
    acc = res.results[0]["o_part"].astype(np.float32)
    for i in range(1, N_CORES):
        acc = acc + res.results[i]["o_part"].astype(np.float32)
    return np.ascontiguousarray(acc.T).reshape(1, SEQ, D_MODEL)

